# revision 1
# baseline (speedup 1.0000x reference)
"""Single-head causal attention (B=4, T=2048, C=1024, H=64) on 8 TRN2 NeuronCores.

Sharding: each batch b is handled by the core pair (2b, 2b+1). Within a pair,
keys/values are split by interleaved 128-row key-tiles (core parity p owns
global key-tiles {2m+p}).  Every core computes q/k/v projections from its
batch's x (host-supplied pre-transposed, columns permuted own-tiles-first so
the SPMD graph is identical on all cores), then causal scores^T, exp, and the
wei@[v|1] partial sums for ALL queries against ITS OWN keys.  The host adds
the two partial outputs of a pair and normalizes (softmax denominator is the
ones-column of the augmented v matmul).

Math notes:
 - scale = C**-0.5 = 1/32 folded into the exp activation's scale.
 - no max-subtraction: scores*scale ~ N(0, 0.25^2) so exp is tiny/safe.
 - compute in bf16 (fp32 PSUM accumulation); partial sums returned fp32.
"""

import os
import sys

sys.path.insert(0, "/opt/trn_rl_repo")

import numpy as np
import ml_dtypes

B, T, C, H = 4, 2048, 1024, 64
NKT = 16  # global 128-row key tiles per batch
OWN = 8  # key tiles per core
QT = 4  # query tiles of 512 (in permuted local order)
SCALE = float(C) ** -0.5

_COMPILED = None
LAST_EXEC_NS = None
LAST_RESULTS = None


def _build_nc(exchange=False):
    import concourse.bass as bass
    import concourse.mybir as mybir
    import concourse.tile as tile
    from concourse import bacc
    from contextlib import ExitStack

    fp32 = mybir.dt.float32
    bf16 = mybir.dt.bfloat16

    # Bacc (not plain Bass): its compile() pipeline lowers multi-wait sync
    # info, inserts gpsimd library loads, etc. — walrus rejects the raw form.
    # detect_race_conditions=False for the exchange build: the sim's rdma race
    # detector flags the cross-core semaphore update, which is the intended
    # synchronization here (wait_ge on a peer-incremented semaphore).
    nc = bacc.Bacc(
        "TRN2",
        target_bir_lowering=False,
        debug=False,
        num_devices=8,
        detect_race_conditions=not exchange,
    )
    # Per-core inputs (host-permuted): xT columns = [my 8 key-tiles | peer 8],
    # each tile 128 consecutive global rows.  With exchange=True only the own
    # half is loaded; peer qT arrives via core-to-core remote DMA.
    xT = nc.declare_dram_parameter("xT", [C, 1024 if exchange else T], fp32, isOutput=False)
    wqk = nc.declare_dram_parameter("wqk", [C, 128], fp32, isOutput=False)
    wv = nc.declare_dram_parameter("wv", [C, H], fp32, isOutput=False)
    # Stair masks, one per (qt, si in 0..1) = slots (2qt, 2qt+1); local-q order.
    masks = nc.declare_dram_parameter("masks", [8, 128, 512], bf16, isOutput=False)
    out_ext = nc.declare_dram_parameter("out", [H + 1, T], fp32, isOutput=True)

    with ExitStack() as ctx:
        tc = ctx.enter_context(tile.TileContext(nc))
        persist = ctx.enter_context(tc.tile_pool(name="persist", bufs=1))
        weipool = ctx.enter_context(tc.tile_pool(name="wei", bufs=2))

        # ---- P0: loads (SWDGE casts fp32->bf16 in flight) ----
        T_own = 1024 if exchange else T
        xT_sb = persist.tile([128, 8, T_own], bf16, tag="xT_sb")
        for c in range(8):
            nc.gpsimd.dma_start(
                out=xT_sb[:, c, :], in_=xT[c * 128 : (c + 1) * 128, :]
            )
        wqk_sb = persist.tile([128, 8, 128], bf16, tag="wqk_sb")
        nc.gpsimd.dma_start(
            out=wqk_sb[:], in_=wqk.rearrange("(c p) j -> p c j", p=128)
        )
        wv_sb = persist.tile([128, 8, H], bf16, tag="wv_sb")
        nc.gpsimd.dma_start(out=wv_sb[:], in_=wv.rearrange("(c p) j -> p c j", p=128))
        mask_sb = persist.tile([128, 8, 512], bf16, tag="mask_sb")
        nc.gpsimd.dma_start(out=mask_sb[:], in_=masks.rearrange("s p y -> p s y"))

        # ---- P1+P2: projections, psum -> sbuf (bf16) ----
        # qT_all local query order per qt: [own(2qt), own(2qt+1), peer(2qt), peer(2qt+1)]
        qT_all = persist.tile([64, T], bf16, tag="qT_all")
        kT_own = persist.tile([64, 1024], bf16, tag="kT_own")
        vT_own = persist.tile([64, 1024], bf16, tag="vT_own")
        import concourse.bass as bass_mod

        def strided_copy(dst_tile, dst_off, src_ap):
            # copy 4 chunks of 256 cols: src chunks at 256*i, dst at 512*i + dst_off
            src = bass_mod.AP(
                tensor=src_ap.tensor,
                offset=src_ap.offset,
                ap=[src_ap.ap[0], [256, 4], [1, 256]],
            )
            d = dst_tile[:, dst_off : dst_off + 1]  # establish tensor/offset
            dst = bass_mod.AP(
                tensor=d.tensor,
                offset=d.offset,
                ap=[d.ap[0], [512, 4], [1, 256]],
            )
            nc.vector.tensor_copy(dst, src)

        if exchange:
            ex_send = persist.tile([128, 512], bf16, tag="ex_send")
            ex_recv = persist.tile([128, 512], bf16, tag="ex_recv")
            rsem = ctx.enter_context(nc.semaphore("rsem"))
            lsem = ctx.enter_context(nc.semaphore("lsem"))

        with tc.tile_pool(name="ps_proj", bufs=2, space="PSUM") as ps_proj:
            # qk over my own columns: out rows 0:64 = qT(own), 64:128 = kT(own)
            qk_ps = ps_proj.tile([128, 1024], fp32, tag="proj", name="qk_ps")
            for c in range(8):
                for n in range(2):
                    nc.tensor.matmul(
                        out=qk_ps[:, n * 512 : (n + 1) * 512],
                        lhsT=wqk_sb[:, c, :],
                        rhs=xT_sb[:, c, n * 512 : (n + 1) * 512],
                        start=(c == 0),
                        stop=(c == 7),
                    )
            strided_copy(qT_all, 0, qk_ps[0:64, :])
            nc.vector.tensor_copy(kT_own[:], qk_ps[64:128, :])

            if exchange:
                # pack my qT [64,1024] -> [128,512] and swap with pair partner
                nc.vector.tensor_copy(ex_send[0:64, :], qk_ps[0:64, 0:512])
                nc.vector.tensor_copy(ex_send[64:128, :], qk_ps[0:64, 512:1024])
            else:
                # q over peer columns
                qp_ps = ps_proj.tile([128, 1024], fp32, tag="proj", name="qp_ps")
                for c in range(8):
                    for n in range(2):
                        nc.tensor.matmul(
                            out=qp_ps[0:64, n * 512 : (n + 1) * 512],
                            lhsT=wqk_sb[:, c, 0:64],
                            rhs=xT_sb[:, c, 1024 + n * 512 : 1024 + (n + 1) * 512],
                            start=(c == 0),
                            stop=(c == 7),
                        )
                strided_copy(qT_all, 256, qp_ps[0:64, :])

            # v over my own columns
            vo_ps = ps_proj.tile([128, 1024], fp32, tag="proj", name="vo_ps")
            for c in range(8):
                for n in range(2):
                    nc.tensor.matmul(
                        out=vo_ps[0:64, n * 512 : (n + 1) * 512],
                        lhsT=wv_sb[:, c, :],
                        rhs=xT_sb[:, c, n * 512 : (n + 1) * 512],
                        start=(c == 0),
                        stop=(c == 7),
                    )
            nc.vector.tensor_copy(vT_own[:], vo_ps[0:64, :])

        if exchange:
            # swap qT halves with the pair partner (tpb XOR 1) over remote DMA,
            # then scatter peer columns into qT_all.  All on gpsimd so the
            # wait_ge -> copies ordering is plain program order.
            def unpack(dst_off, src_rows):
                d = qT_all[:, dst_off : dst_off + 1]
                dst = bass_mod.AP(
                    tensor=d.tensor, offset=d.offset, ap=[d.ap[0], [512, 2], [1, 256]]
                )
                s = ex_recv[src_rows * 64 : src_rows * 64 + 64, :]
                src = bass_mod.AP(
                    tensor=s.tensor, offset=s.offset, ap=[s.ap[0], [256, 2], [1, 256]]
                )
                nc.vector.tensor_copy(dst, src)

            with tc.tile_critical():
                # clear BEFORE our trigger: the peer's update cannot arrive
                # until after its own (symmetric) trigger, so clearing here
                # cannot wipe it; also makes the NEFF re-executable.
                # (Bacc.compile inserts the remote_dma gpsimd library load.)
                nc.gpsimd.sem_clear(rsem)
                nc.gpsimd.sem_clear(lsem)
                nc.gpsimd.remote_dma_broadcast(
                    out_ap=ex_recv[:],
                    in_ap=ex_send[:],
                    remote_sem=rsem,
                    local_sem=lsem,
                    rdests=[(0, 1)] + [None] * 7,
                )
                nc.gpsimd.trigger_dma(count=1)
                nc.vector.wait_ge(rsem, 2)
                unpack(256, 0)
                unpack(256 + 1024, 1)

        # ---- P3: v row-layout tiles with ones column ----
        # PE-mode transpose (sbuf->psum via identity), not DMA transpose —
        # the xbar transpose path hung on hardware here.
        from concourse.masks import make_identity

        v_sb = persist.tile([128, 8, H + 1], bf16, tag="v_sb")
        ident = persist.tile([128, 128], bf16, tag="ident")
        make_identity(nc, ident[:])
        with tc.tile_pool(name="ps_vt", bufs=2, space="PSUM") as ps_vt:
            for s in range(8):
                nc.vector.memset(v_sb[:, s, H : H + 1], 1.0)
                vt_ps = ps_vt.tile([128, H], bf16, tag="vt", name="vt_ps")
                nc.tensor.transpose(
                    vt_ps[:], vT_own[:, s * 128 : (s + 1) * 128], ident[0:64, 0:64]
                )
                nc.vector.tensor_copy(v_sb[:, s, 0:H], vt_ps[:])

        # ---- P4: attention (scores^T -> exp -> stair mask -> PV) ----
        out_sb = persist.tile([H + 1, T], fp32, tag="out_sb")
        ps_sc4 = ctx.enter_context(tc.tile_pool(name="ps_sc4", bufs=1, space="PSUM"))
        ps_sc2 = ctx.enter_context(tc.tile_pool(name="ps_sc2", bufs=1, space="PSUM"))
        ps_pv = ctx.enter_context(tc.tile_pool(name="ps_pv", bufs=2, space="PSUM"))

        def do_scores(ps, ps_off, qt, s):
            nc.tensor.matmul(
                out=ps[:, ps_off : ps_off + 512],
                lhsT=kT_own[:, s * 128 : (s + 1) * 128],
                rhs=qT_all[:, qt * 512 : (qt + 1) * 512],
                start=True,
                stop=True,
                skip_group_check=True,
            )

        for qt in range(QT):
            nkt = 2 * qt + 2
            wei = weipool.tile([128, 4096], bf16, tag="wei")
            # groups of score slots sharing one psum tile + one exp instruction
            groups = []
            s0 = 0
            while s0 < nkt:
                g = min(4 if nkt - s0 >= 4 else 2, nkt - s0)
                groups.append((s0, g))
                s0 += g
            for g0, glen in groups:
                if glen > 2:
                    ps = ps_sc4.tile([128, 2048], fp32, tag="sc4", name="sc4")
                else:
                    ps = ps_sc2.tile([128, 1024], fp32, tag="sc2", name="sc2")
                for i in range(glen):
                    do_scores(ps, i * 512, qt, g0 + i)
                nc.scalar.activation(
                    out=wei[:, g0 * 512 : (g0 + glen) * 512],
                    in_=ps[:, 0 : glen * 512],
                    func=mybir.ActivationFunctionType.Exp,
                    scale=SCALE,
                )
            # stair masks on the last two slots (2qt, 2qt+1)
            for si in range(2):
                slot = 2 * qt + si
                nc.vector.tensor_mul(
                    out=wei[:, slot * 512 : (slot + 1) * 512],
                    in0=wei[:, slot * 512 : (slot + 1) * 512],
                    in1=mask_sb[:, slot, :],
                )
            # PV accumulate over slots
            pv = ps_pv.tile([H + 1, 512], fp32, tag="pv")
            for s in range(nkt):
                nc.tensor.matmul(
                    out=pv[:],
                    lhsT=v_sb[:, s, :],
                    rhs=wei[:, s * 512 : (s + 1) * 512],
                    start=(s == 0),
                    stop=(s == nkt - 1),
                    skip_group_check=True,
                )
            nc.vector.tensor_copy(out_sb[:, qt * 512 : (qt + 1) * 512], pv[:])

        # ---- P5: store partial sums ----
        nc.sync.dma_start(out=out_ext[:], in_=out_sb[:])

    nc.compile()
    return nc


def _local_q_perm(p):
    """global query index for each local column (length T)."""
    perm = np.empty(T, dtype=np.int64)
    for qt in range(QT):
        blk_kts = [4 * qt + p, 4 * qt + 2 + p, 4 * qt + (1 - p), 4 * qt + 2 + (1 - p)]
        for i, kt in enumerate(blk_kts):
            lo = qt * 512 + i * 128
            perm[lo : lo + 128] = np.arange(kt * 128, kt * 128 + 128)
    return perm


def _build_masks(p):
    """[8,128,512] bf16: slot (qt, si) masks score block of own key-tile
    own_kts[2qt+si] vs the local-order query tile qt."""
    own_kts = [2 * m + p for m in range(8)]
    perm = _local_q_perm(p)
    masks = np.zeros((8, 128, 512), dtype=np.float32)
    for qt in range(QT):
        qg = perm[qt * 512 : (qt + 1) * 512]  # global query index per local col
        for si in range(2):
            kt = own_kts[2 * qt + si]
            keys = np.arange(kt * 128, kt * 128 + 128)
            masks[2 * qt + si] = (keys[:, None] <= qg[None, :]).astype(np.float32)
    return masks.astype(ml_dtypes.bfloat16)


def _make_in_maps(x, Wq, Wk, Wv, exchange=False):
    wqk = np.ascontiguousarray(np.concatenate([Wq, Wk], axis=1))
    wv = np.ascontiguousarray(Wv)
    in_maps = []
    for c in range(8):
        b, p = c // 2, c % 2
        own_kts = [2 * m + p for m in range(8)]
        peer_kts = [2 * m + (1 - p) for m in range(8)]
        kts = own_kts if exchange else own_kts + peer_kts
        rows = np.concatenate([np.arange(kt * 128, kt * 128 + 128) for kt in kts])
        xT_perm = np.ascontiguousarray(x[b][rows].T)  # [C, 1024 or T]
        in_maps.append(
            {"xT": xT_perm, "wqk": wqk, "wv": wv, "masks": _build_masks(p)}
        )
    return in_maps


def _combine(per_core_out):
    """per_core_out: list of 8 arrays [H+1, T] (local query order) -> [B,T,H]."""
    out = np.empty((B, T, H), dtype=np.float32)
    for b in range(B):
        S = None
        for p in range(2):
            P_local = np.asarray(per_core_out[2 * b + p], dtype=np.float32)
            perm = _local_q_perm(p)
            P_glob = np.empty_like(P_local)
            P_glob[:, perm] = P_local
            S = P_glob if S is None else S + P_glob
        out[b] = (S[0:H, :] / S[H : H + 1, :]).T
    return out


EXCHANGE = os.environ.get("BASS_KERNEL_EXCHANGE", "0") == "1"


def kernel(x, Wq, Wk, Wv):
    global _COMPILED, LAST_EXEC_NS, LAST_RESULTS
    from concourse.bass_utils import run_bass_kernel_spmd

    x = np.ascontiguousarray(np.asarray(x, dtype=np.float32))
    Wq = np.asarray(Wq, dtype=np.float32)
    Wk = np.asarray(Wk, dtype=np.float32)
    Wv = np.asarray(Wv, dtype=np.float32)

    if _COMPILED is None:
        _COMPILED = _build_nc(exchange=EXCHANGE)
    nc = _COMPILED

    in_maps = _make_in_maps(x, Wq, Wk, Wv, exchange=EXCHANGE)
    trace = os.environ.get("BASS_KERNEL_TRACE", "0") == "1"
    res = run_bass_kernel_spmd(nc, in_maps, core_ids=list(range(8)), trace=trace)
    LAST_EXEC_NS = getattr(res, "exec_time_ns", None)
    LAST_RESULTS = res
    return _combine([res.results[c]["out"] for c in range(8)])



# revision 4
# speedup vs baseline: 1.2051x; 1.2051x over previous
"""Single-head causal attention (B=4, T=2048, C=1024, H=64) on 8 TRN2 NeuronCores.

Sharding: each batch b is handled by the core pair (2b, 2b+1). Within a pair,
keys/values are split by interleaved 128-row key-tiles (core parity p owns
global key-tiles {2m+p}).  Every core computes q/k/v projections from its
batch's x (host-supplied pre-transposed, columns permuted own-tiles-first so
the SPMD graph is identical on all cores), then causal scores^T, exp, and the
wei@[v|1] partial sums for ALL queries against ITS OWN keys.  The host adds
the two partial outputs of a pair and normalizes (softmax denominator is the
ones-column of the augmented v matmul).

Math notes:
 - scale = C**-0.5 = 1/32 folded into the exp activation's scale.
 - no max-subtraction: scores*scale ~ N(0, 0.25^2) so exp is tiny/safe.
 - compute in bf16 (fp32 PSUM accumulation); partial sums returned fp32.
"""

import os
import sys

sys.path.insert(0, "/opt/trn_rl_repo")

import numpy as np
import ml_dtypes

B, T, C, H = 4, 2048, 1024, 64
NKT = 16  # global 128-row key tiles per batch
OWN = 8  # key tiles per core
QT = 4  # query tiles of 512 (in permuted local order)
SCALE = float(C) ** -0.5

_COMPILED = None
LAST_EXEC_NS = None
LAST_RESULTS = None


def _build_nc(exchange=False):
    import concourse.bass as bass
    import concourse.mybir as mybir
    import concourse.tile as tile
    from concourse import bacc
    from contextlib import ExitStack

    fp32 = mybir.dt.float32
    bf16 = mybir.dt.bfloat16

    # Bacc (not plain Bass): its compile() pipeline lowers multi-wait sync
    # info, inserts gpsimd library loads, etc. — walrus rejects the raw form.
    # detect_race_conditions=False for the exchange build: the sim's rdma race
    # detector flags the cross-core semaphore update, which is the intended
    # synchronization here (wait_ge on a peer-incremented semaphore).
    nc = bacc.Bacc(
        "TRN2",
        target_bir_lowering=False,
        debug=False,
        num_devices=8,
        detect_race_conditions=not exchange,
    )
    # Per-core inputs (host-permuted + host-cast to bf16): xT columns =
    # [my 8 key-tiles | peer 8], each tile 128 consecutive global rows.
    # With exchange=True only the own half is loaded; peer qT arrives via
    # core-to-core remote DMA.  Weights/masks are host-pre-permuted into the
    # SBUF layout so every HWDGE descriptor is a contiguous 2KB+ row.
    xT = nc.declare_dram_parameter("xT", [C, 1024 if exchange else T], bf16, isOutput=False)
    wqk = nc.declare_dram_parameter("wqk", [128, 8 * 128], bf16, isOutput=False)
    wv = nc.declare_dram_parameter("wv", [128, 8 * H], bf16, isOutput=False)
    # Stair masks, one per (qt, si in 0..1) = slots (2qt, 2qt+1); local-q order.
    masks = nc.declare_dram_parameter("masks", [128, 8 * 512], bf16, isOutput=False)
    out_ext = nc.declare_dram_parameter("out", [H + 1, T], fp32, isOutput=True)

    with ExitStack() as ctx:
        tc = ctx.enter_context(tile.TileContext(nc))
        persist = ctx.enter_context(tc.tile_pool(name="persist", bufs=1))
        weipool = ctx.enter_context(tc.tile_pool(name="wei", bufs=2))

        # ---- P0: loads (bf16 host-cast; HWDGE on sync/scalar queues) ----
        T_own = 1024 if exchange else T
        xT_sb = persist.tile([128, 8, T_own], bf16, tag="xT_sb")
        for c in range(8):
            nc.sync.dma_start(
                out=xT_sb[:, c, :], in_=xT[c * 128 : (c + 1) * 128, :]
            )
        wqk_sb = persist.tile([128, 8, 128], bf16, tag="wqk_sb")
        nc.scalar.dma_start(out=wqk_sb[:], in_=wqk[:])
        wv_sb = persist.tile([128, 8, H], bf16, tag="wv_sb")
        nc.scalar.dma_start(out=wv_sb[:], in_=wv[:])
        mask_sb = persist.tile([128, 8, 512], bf16, tag="mask_sb")
        nc.scalar.dma_start(out=mask_sb[:], in_=masks[:])

        # ---- P1+P2: projections, psum -> sbuf (bf16) ----
        # qT_all local query order per qt: [own(2qt), own(2qt+1), peer(2qt), peer(2qt+1)]
        qT_all = persist.tile([64, T], bf16, tag="qT_all")
        kT_own = persist.tile([64, 1024], bf16, tag="kT_own")
        vT_own = persist.tile([64, 1024], bf16, tag="vT_own")
        import concourse.bass as bass_mod

        def strided_copy(dst_tile, dst_off, src_ap):
            # copy 4 chunks of 256 cols: src chunks at 256*i, dst at 512*i + dst_off
            src = bass_mod.AP(
                tensor=src_ap.tensor,
                offset=src_ap.offset,
                ap=[src_ap.ap[0], [256, 4], [1, 256]],
            )
            d = dst_tile[:, dst_off : dst_off + 1]  # establish tensor/offset
            dst = bass_mod.AP(
                tensor=d.tensor,
                offset=d.offset,
                ap=[d.ap[0], [512, 4], [1, 256]],
            )
            nc.vector.tensor_copy(dst, src)

        if exchange:
            ex_send = persist.tile([128, 512], bf16, tag="ex_send")
            ex_recv = persist.tile([128, 512], bf16, tag="ex_recv")
            rsem = ctx.enter_context(nc.semaphore("rsem"))
            lsem = ctx.enter_context(nc.semaphore("lsem"))

        with tc.tile_pool(name="ps_proj", bufs=2, space="PSUM") as ps_proj:
            # qk over my own columns: out rows 0:64 = qT(own), 64:128 = kT(own)
            qk_ps = ps_proj.tile([128, 1024], fp32, tag="proj", name="qk_ps")
            for c in range(8):
                for n in range(2):
                    nc.tensor.matmul(
                        out=qk_ps[:, n * 512 : (n + 1) * 512],
                        lhsT=wqk_sb[:, c, :],
                        rhs=xT_sb[:, c, n * 512 : (n + 1) * 512],
                        start=(c == 0),
                        stop=(c == 7),
                    )
            strided_copy(qT_all, 0, qk_ps[0:64, :])
            nc.vector.tensor_copy(kT_own[:], qk_ps[64:128, :])

            if exchange:
                # pack my qT [64,1024] -> [128,512] and swap with pair partner
                nc.vector.tensor_copy(ex_send[0:64, :], qk_ps[0:64, 0:512])
                nc.vector.tensor_copy(ex_send[64:128, :], qk_ps[0:64, 512:1024])
            else:
                # q over peer columns
                qp_ps = ps_proj.tile([128, 1024], fp32, tag="proj", name="qp_ps")
                for c in range(8):
                    for n in range(2):
                        nc.tensor.matmul(
                            out=qp_ps[0:64, n * 512 : (n + 1) * 512],
                            lhsT=wqk_sb[:, c, 0:64],
                            rhs=xT_sb[:, c, 1024 + n * 512 : 1024 + (n + 1) * 512],
                            start=(c == 0),
                            stop=(c == 7),
                        )
                strided_copy(qT_all, 256, qp_ps[0:64, :])

            # v over my own columns
            vo_ps = ps_proj.tile([128, 1024], fp32, tag="proj", name="vo_ps")
            for c in range(8):
                for n in range(2):
                    nc.tensor.matmul(
                        out=vo_ps[0:64, n * 512 : (n + 1) * 512],
                        lhsT=wv_sb[:, c, :],
                        rhs=xT_sb[:, c, n * 512 : (n + 1) * 512],
                        start=(c == 0),
                        stop=(c == 7),
                    )
            nc.vector.tensor_copy(vT_own[:], vo_ps[0:64, :])

        if exchange:
            # swap qT halves with the pair partner (tpb XOR 1) over remote DMA,
            # then scatter peer columns into qT_all.  All on gpsimd so the
            # wait_ge -> copies ordering is plain program order.
            def unpack(dst_off, src_rows):
                d = qT_all[:, dst_off : dst_off + 1]
                dst = bass_mod.AP(
                    tensor=d.tensor, offset=d.offset, ap=[d.ap[0], [512, 2], [1, 256]]
                )
                s = ex_recv[src_rows * 64 : src_rows * 64 + 64, :]
                src = bass_mod.AP(
                    tensor=s.tensor, offset=s.offset, ap=[s.ap[0], [256, 2], [1, 256]]
                )
                nc.vector.tensor_copy(dst, src)

            with tc.tile_critical():
                # clear BEFORE our trigger: the peer's update cannot arrive
                # until after its own (symmetric) trigger, so clearing here
                # cannot wipe it; also makes the NEFF re-executable.
                # (Bacc.compile inserts the remote_dma gpsimd library load.)
                nc.gpsimd.sem_clear(rsem)
                nc.gpsimd.sem_clear(lsem)
                nc.gpsimd.remote_dma_broadcast(
                    out_ap=ex_recv[:],
                    in_ap=ex_send[:],
                    remote_sem=rsem,
                    local_sem=lsem,
                    rdests=[(0, 1)] + [None] * 7,
                )
                nc.gpsimd.trigger_dma(count=1)
                nc.vector.wait_ge(rsem, 2)
                unpack(256, 0)
                unpack(256 + 1024, 1)

        # ---- P3: v row-layout tiles with ones column ----
        # PE-mode transpose (sbuf->psum via identity), not DMA transpose —
        # the xbar transpose path hung on hardware here.
        from concourse.masks import make_identity

        v_sb = persist.tile([128, 8, H + 1], bf16, tag="v_sb")
        ident = persist.tile([128, 128], bf16, tag="ident")
        make_identity(nc, ident[:])
        with tc.tile_pool(name="ps_vt", bufs=2, space="PSUM") as ps_vt:
            for s in range(8):
                nc.vector.memset(v_sb[:, s, H : H + 1], 1.0)
                vt_ps = ps_vt.tile([128, H], bf16, tag="vt", name="vt_ps")
                nc.tensor.transpose(
                    vt_ps[:], vT_own[:, s * 128 : (s + 1) * 128], ident[0:64, 0:64]
                )
                nc.vector.tensor_copy(v_sb[:, s, 0:H], vt_ps[:])

        # ---- P4: attention (scores^T -> exp -> stair mask -> PV) ----
        out_sb = persist.tile([H + 1, T], fp32, tag="out_sb")
        ps_sc4 = ctx.enter_context(tc.tile_pool(name="ps_sc4", bufs=1, space="PSUM"))
        ps_sc2 = ctx.enter_context(tc.tile_pool(name="ps_sc2", bufs=1, space="PSUM"))
        ps_pv = ctx.enter_context(tc.tile_pool(name="ps_pv", bufs=2, space="PSUM"))

        def do_scores(ps, ps_off, qt, s):
            nc.tensor.matmul(
                out=ps[:, ps_off : ps_off + 512],
                lhsT=kT_own[:, s * 128 : (s + 1) * 128],
                rhs=qT_all[:, qt * 512 : (qt + 1) * 512],
                start=True,
                stop=True,
                skip_group_check=True,
            )

        for qt in range(QT):
            nkt = 2 * qt + 2
            wei = weipool.tile([128, 4096], bf16, tag="wei")
            # groups of score slots sharing one psum tile + one exp instruction
            groups = []
            s0 = 0
            while s0 < nkt:
                g = min(4 if nkt - s0 >= 4 else 2, nkt - s0)
                groups.append((s0, g))
                s0 += g
            for g0, glen in groups:
                if glen > 2:
                    ps = ps_sc4.tile([128, 2048], fp32, tag="sc4", name="sc4")
                else:
                    ps = ps_sc2.tile([128, 1024], fp32, tag="sc2", name="sc2")
                for i in range(glen):
                    do_scores(ps, i * 512, qt, g0 + i)
                nc.scalar.activation(
                    out=wei[:, g0 * 512 : (g0 + glen) * 512],
                    in_=ps[:, 0 : glen * 512],
                    func=mybir.ActivationFunctionType.Exp,
                    scale=SCALE,
                )
            # stair masks on the last two slots (2qt, 2qt+1)
            for si in range(2):
                slot = 2 * qt + si
                nc.vector.tensor_mul(
                    out=wei[:, slot * 512 : (slot + 1) * 512],
                    in0=wei[:, slot * 512 : (slot + 1) * 512],
                    in1=mask_sb[:, slot, :],
                )
            # PV accumulate over slots
            pv = ps_pv.tile([H + 1, 512], fp32, tag="pv")
            for s in range(nkt):
                nc.tensor.matmul(
                    out=pv[:],
                    lhsT=v_sb[:, s, :],
                    rhs=wei[:, s * 512 : (s + 1) * 512],
                    start=(s == 0),
                    stop=(s == nkt - 1),
                    skip_group_check=True,
                )
            nc.vector.tensor_copy(out_sb[:, qt * 512 : (qt + 1) * 512], pv[:])

        # ---- P5: store partial sums ----
        nc.sync.dma_start(out=out_ext[:], in_=out_sb[:])

    nc.compile()
    return nc


def _local_q_perm(p):
    """global query index for each local column (length T)."""
    perm = np.empty(T, dtype=np.int64)
    for qt in range(QT):
        blk_kts = [4 * qt + p, 4 * qt + 2 + p, 4 * qt + (1 - p), 4 * qt + 2 + (1 - p)]
        for i, kt in enumerate(blk_kts):
            lo = qt * 512 + i * 128
            perm[lo : lo + 128] = np.arange(kt * 128, kt * 128 + 128)
    return perm


def _build_masks(p):
    """[8,128,512] bf16: slot (qt, si) masks score block of own key-tile
    own_kts[2qt+si] vs the local-order query tile qt."""
    own_kts = [2 * m + p for m in range(8)]
    perm = _local_q_perm(p)
    masks = np.zeros((8, 128, 512), dtype=np.float32)
    for qt in range(QT):
        qg = perm[qt * 512 : (qt + 1) * 512]  # global query index per local col
        for si in range(2):
            kt = own_kts[2 * qt + si]
            keys = np.arange(kt * 128, kt * 128 + 128)
            masks[2 * qt + si] = (keys[:, None] <= qg[None, :]).astype(np.float32)
    return masks.astype(ml_dtypes.bfloat16)


def _make_in_maps(x, Wq, Wk, Wv, exchange=False):
    bf16 = ml_dtypes.bfloat16
    # [C, 128+H] -> SBUF layout [p=128, c=8, j]: row p holds chunk-c rows c*128+p
    wqk = np.concatenate([Wq, Wk], axis=1).reshape(8, 128, 128)
    wqk = np.ascontiguousarray(wqk.transpose(1, 0, 2).reshape(128, 8 * 128)).astype(bf16)
    wv = Wv.reshape(8, 128, H)
    wv = np.ascontiguousarray(wv.transpose(1, 0, 2).reshape(128, 8 * H)).astype(bf16)
    in_maps = []
    for c in range(8):
        b, p = c // 2, c % 2
        own_kts = [2 * m + p for m in range(8)]
        peer_kts = [2 * m + (1 - p) for m in range(8)]
        kts = own_kts if exchange else own_kts + peer_kts
        rows = np.concatenate([np.arange(kt * 128, kt * 128 + 128) for kt in kts])
        xT_perm = np.ascontiguousarray(x[b][rows].T.astype(bf16))  # [C, 1024 or T]
        # masks [8,128,512] -> [128, 8*512]
        m = np.ascontiguousarray(
            _build_masks(p).transpose(1, 0, 2).reshape(128, 8 * 512)
        )
        in_maps.append({"xT": xT_perm, "wqk": wqk, "wv": wv, "masks": m})
    return in_maps


def _combine(per_core_out):
    """per_core_out: list of 8 arrays [H+1, T] (local query order) -> [B,T,H]."""
    out = np.empty((B, T, H), dtype=np.float32)
    for b in range(B):
        S = None
        for p in range(2):
            P_local = np.asarray(per_core_out[2 * b + p], dtype=np.float32)
            perm = _local_q_perm(p)
            P_glob = np.empty_like(P_local)
            P_glob[:, perm] = P_local
            S = P_glob if S is None else S + P_glob
        out[b] = (S[0:H, :] / S[H : H + 1, :]).T
    return out


EXCHANGE = os.environ.get("BASS_KERNEL_EXCHANGE", "0") == "1"


def kernel(x, Wq, Wk, Wv):
    global _COMPILED, LAST_EXEC_NS, LAST_RESULTS
    from concourse.bass_utils import run_bass_kernel_spmd

    x = np.ascontiguousarray(np.asarray(x, dtype=np.float32))
    Wq = np.asarray(Wq, dtype=np.float32)
    Wk = np.asarray(Wk, dtype=np.float32)
    Wv = np.asarray(Wv, dtype=np.float32)

    if _COMPILED is None:
        _COMPILED = _build_nc(exchange=EXCHANGE)
    nc = _COMPILED

    in_maps = _make_in_maps(x, Wq, Wk, Wv, exchange=EXCHANGE)
    trace = os.environ.get("BASS_KERNEL_TRACE", "0") == "1"
    res = run_bass_kernel_spmd(nc, in_maps, core_ids=list(range(8)), trace=trace)
    LAST_EXEC_NS = getattr(res, "exec_time_ns", None)
    LAST_RESULTS = res
    return _combine([res.results[c]["out"] for c in range(8)])



# revision 9
# speedup vs baseline: 1.2989x; 1.0779x over previous
"""Single-head causal attention (B=4, T=2048, C=1024, H=64) on 8 TRN2 NeuronCores.

Sharding: each batch b is handled by the core pair (2b, 2b+1). Within a pair,
keys/values are split by interleaved 128-row key-tiles (core parity p owns
global key-tiles {2m+p}).  Every core computes q/k/v projections from its own
1024 x-columns (host-supplied pre-transposed bf16, own-tiles-first), swaps qT
halves with its pair partner over core-to-core remote DMA, then computes
causal scores^T, exp, and the wei@[v|1] partial sums for ALL queries against
ITS OWN keys.  The host adds the two partial outputs of a pair and normalizes
(softmax denominator is the ones-column of the augmented v matmul).

Math notes:
 - scale = C**-0.5 = 1/32 folded into the exp activation's scale.
 - no max-subtraction: scores*scale ~ N(0, 0.25^2) so exp is tiny/safe.
 - compute in bf16 (fp32 PSUM accumulation); partial sums returned fp32.
 - stair masks are qt-independent in local column order: only 2 slot masks
   [128, 512] are needed ([TRI,1,c,1] and [0,TRI,0,c], c = parity coeff).
"""

import os
import sys

sys.path.insert(0, "/opt/trn_rl_repo")

import numpy as np
import ml_dtypes

B, T, C, H = 4, 2048, 1024, 64
NKT = 16  # global 128-row key tiles per batch
OWN = 8  # key tiles per core
QT = 4  # query tiles of 512 (in permuted local order)
SCALE = float(C) ** -0.5

_COMPILED = None
LAST_EXEC_NS = None
LAST_RESULTS = None


def _build_nc(exchange=True):
    import concourse.bass as bass
    import concourse.mybir as mybir
    import concourse.tile as tile
    from concourse import bacc
    from concourse.masks import make_identity
    from contextlib import ExitStack

    fp32 = mybir.dt.float32
    bf16 = mybir.dt.bfloat16

    # Bacc (not plain Bass): its compile() pipeline lowers multi-wait sync
    # info, inserts gpsimd library loads, etc.  detect_race_conditions=False
    # for the exchange build: the sim's rdma race detector flags the
    # cross-core semaphore update, which is the intended synchronization.
    nc = bacc.Bacc(
        "TRN2",
        target_bir_lowering=False,
        debug=False,
        num_devices=8,
        detect_race_conditions=not exchange,
    )
    T_own = 1024 if exchange else T
    xT = nc.declare_dram_parameter("xT", [C, T_own], bf16, isOutput=False)
    wqk = nc.declare_dram_parameter("wqk", [128, 8 * 128], bf16, isOutput=False)
    wv = nc.declare_dram_parameter("wv", [128, 8 * H], bf16, isOutput=False)
    # Two slot masks [si, 128, 512] flattened: si=0 for slot 2qt, si=1 for 2qt+1
    masks = nc.declare_dram_parameter("masks", [128, 2 * 512], bf16, isOutput=False)
    out_ext = nc.declare_dram_parameter("out", [H + 1, T], fp32, isOutput=True)

    import concourse.bass as bass_mod

    with ExitStack() as ctx:
        tc = ctx.enter_context(tile.TileContext(nc))
        persist = ctx.enter_context(tc.tile_pool(name="persist", bufs=1))
        weipool = ctx.enter_context(tc.tile_pool(name="wei", bufs=2))

        # ---- P0: loads (bf16 host-cast; HWDGE) ----
        xT_sb = persist.tile([128, 8, T_own], bf16, tag="xT_sb")
        for c in range(8):
            nc.sync.dma_start(out=xT_sb[:, c, :], in_=xT[c * 128 : (c + 1) * 128, :])
        wqk_sb = persist.tile([128, 8, 128], bf16, tag="wqk_sb")
        nc.scalar.dma_start(out=wqk_sb[:], in_=wqk[:])
        wv_sb = persist.tile([128, 8, H], bf16, tag="wv_sb")
        nc.scalar.dma_start(out=wv_sb[:], in_=wv[:])
        mask_sb = persist.tile([128, 2 * 512], bf16, tag="mask_sb")
        nc.scalar.dma_start(out=mask_sb[:], in_=masks[:])

        # persistent SBUF tensors
        qT_all = persist.tile([64, T], bf16, tag="qT_all")
        kT_own = persist.tile([64, 1024], bf16, tag="kT_own")
        vT_own = persist.tile([64, 1024], bf16, tag="vT_own")
        v_sb = persist.tile([128, 8, H + 1], bf16, tag="v_sb")
        ident = persist.tile([128, 128], bf16, tag="ident")
        make_identity(nc, ident[:])
        nc.gpsimd.memset(v_sb[:, :, H : H + 1], 1.0)

        if exchange:
            ex_send = persist.tile([128, 512], bf16, tag="ex_send")
            ex_recv = persist.tile([128, 512], bf16, tag="ex_recv")
            rsem = ctx.enter_context(nc.semaphore("rsem"))
            lsem = ctx.enter_context(nc.semaphore("lsem"))

        def strided_pair_copy(engine, dst_tile, dst_off, src_ap, n=2):
            """copy n chunks of 256 cols: src chunk i at 256*i, dst at 512*i+dst_off"""
            src = bass_mod.AP(
                tensor=src_ap.tensor,
                offset=src_ap.offset,
                ap=[src_ap.ap[0], [256, n], [1, 256]],
            )
            d = dst_tile[:, dst_off : dst_off + 1]
            dst = bass_mod.AP(
                tensor=d.tensor, offset=d.offset, ap=[d.ap[0], [512, n], [1, 256]]
            )
            engine.tensor_copy(dst, src)

        # ---- P1: projections (psum), interleaved per K-chunk for DMA overlap ----
        proj_ctx = ExitStack()
        ps_qk = proj_ctx.enter_context(tc.tile_pool(name="ps_qk", bufs=1, space="PSUM"))
        ps_vo = proj_ctx.enter_context(tc.tile_pool(name="ps_vo", bufs=1, space="PSUM"))
        qk_ps = ps_qk.tile([128, 1024], fp32, tag="qk", name="qk_ps")
        vo_ps = ps_vo.tile([64, 1024], fp32, tag="vo", name="vo_ps")
        if not exchange:
            ps_qp = proj_ctx.enter_context(tc.tile_pool(name="ps_qp", bufs=1, space="PSUM"))
            qp_ps = ps_qp.tile([64, 1024], fp32, tag="qp", name="qp_ps")
        for c in range(8):
            for n in range(2):
                nc.tensor.matmul(
                    out=qk_ps[:, n * 512 : (n + 1) * 512],
                    lhsT=wqk_sb[:, c, :],
                    rhs=xT_sb[:, c, n * 512 : (n + 1) * 512],
                    start=(c == 0),
                    stop=(c == 7),
                )
            for n in range(2):
                nc.tensor.matmul(
                    out=vo_ps[:, n * 512 : (n + 1) * 512],
                    lhsT=wv_sb[:, c, :],
                    rhs=xT_sb[:, c, n * 512 : (n + 1) * 512],
                    start=(c == 0),
                    stop=(c == 7),
                )
            if not exchange:
                for n in range(2):
                    nc.tensor.matmul(
                        out=qp_ps[:, n * 512 : (n + 1) * 512],
                        lhsT=wqk_sb[:, c, 0:64],
                        rhs=xT_sb[:, c, 1024 + n * 512 : 1024 + (n + 1) * 512],
                        start=(c == 0),
                        stop=(c == 7),
                    )

        # ---- P2: psum -> sbuf (bf16 casts), spread across Act/DVE ----
        for h in range(2):
            cols = slice(h * 512, (h + 1) * 512)
            if exchange:
                # pack my qT half h -> ex_send rows [64h : 64h+64]
                nc.scalar.copy(ex_send[h * 64 : (h + 1) * 64, :], qk_ps[0:64, cols])
            nc.scalar.copy(kT_own[:, cols], qk_ps[64:128, cols])
            nc.scalar.copy(vT_own[:, cols], vo_ps[:, cols])
            # own qT half h -> qT_all local blocks (2h)*512, (2h+1)*512 (+0 offset)
            strided_pair_copy(nc.vector, qT_all, (2 * h) * 512, qk_ps[0:64, cols])
        if not exchange:
            strided_pair_copy(nc.vector, qT_all, 256, qp_ps[0:64, 0:1024], n=4)
        proj_ctx.close()

        # ---- P2b: swap qT halves with pair partner (tpb XOR 1) ----
        if exchange:
            def unpack(h):
                d = qT_all[:, 256 + (2 * h) * 512 : 256 + (2 * h) * 512 + 1]
                dst = bass_mod.AP(
                    tensor=d.tensor, offset=d.offset, ap=[d.ap[0], [512, 2], [1, 256]]
                )
                s = ex_recv[h * 64 : (h + 1) * 64, :]
                src = bass_mod.AP(
                    tensor=s.tensor, offset=s.offset, ap=[s.ap[0], [256, 2], [1, 256]]
                )
                nc.vector.tensor_copy(dst, src)

            with tc.tile_critical():
                # clear BEFORE our trigger: the peer's update cannot arrive
                # until after its own (symmetric) trigger, so clearing here
                # cannot wipe it; also makes the NEFF re-executable.
                nc.gpsimd.sem_clear(rsem)
                nc.gpsimd.sem_clear(lsem)
                nc.gpsimd.remote_dma_broadcast(
                    out_ap=ex_recv[:],
                    in_ap=ex_send[:],
                    remote_sem=rsem,
                    local_sem=lsem,
                    rdests=[(0, 1)] + [None] * 7,
                )
                nc.gpsimd.trigger_dma(count=1)
                nc.vector.wait_ge(rsem, 2)
                unpack(0)
                unpack(1)

        # ---- P3: v row-layout tiles (PE-mode transpose) + ones column ----
        with tc.tile_pool(name="ps_vt", bufs=2, space="PSUM") as ps_vt:
            for s in range(8):
                vt_ps = ps_vt.tile([128, H], bf16, tag="vt", name="vt_ps")
                nc.tensor.transpose(
                    vt_ps[:], vT_own[:, s * 128 : (s + 1) * 128], ident[0:64, 0:64]
                )
                nc.vector.tensor_copy(v_sb[:, s, 0:H], vt_ps[:])

        # ---- P4: attention (scores^T -> exp -> stair mask -> PV) ----
        out_sb = persist.tile([H + 1, T], fp32, tag="out_sb")
        ps_sc4 = ctx.enter_context(tc.tile_pool(name="ps_sc4", bufs=1, space="PSUM"))
        ps_sc2 = ctx.enter_context(tc.tile_pool(name="ps_sc2", bufs=1, space="PSUM"))
        ps_pv = ctx.enter_context(tc.tile_pool(name="ps_pv", bufs=2, space="PSUM"))

        def do_scores(ps, ps_off, qt, s):
            nc.tensor.matmul(
                out=ps[:, ps_off : ps_off + 512],
                lhsT=kT_own[:, s * 128 : (s + 1) * 128],
                rhs=qT_all[:, qt * 512 : (qt + 1) * 512],
                start=True,
                stop=True,
                skip_group_check=True,
            )

        for qt in range(QT):
            nkt = 2 * qt + 2
            wei = weipool.tile([128, 4096], bf16, tag="wei")
            groups = []
            s0 = 0
            while s0 < nkt:
                g = min(4 if nkt - s0 >= 4 else 2, nkt - s0)
                groups.append((s0, g))
                s0 += g
            for g0, glen in groups:
                if glen > 2:
                    ps = ps_sc4.tile([128, 2048], fp32, tag="sc4", name="sc4")
                else:
                    ps = ps_sc2.tile([128, 1024], fp32, tag="sc2", name="sc2")
                for i in range(glen):
                    do_scores(ps, i * 512, qt, g0 + i)
                nc.scalar.activation(
                    out=wei[:, g0 * 512 : (g0 + glen) * 512],
                    in_=ps[:, 0 : glen * 512],
                    func=mybir.ActivationFunctionType.Exp,
                    scale=SCALE,
                )
            # stair masks: slots (2qt, 2qt+1) = last 1024 cols, one fused mul
            nc.vector.tensor_mul(
                out=wei[:, 2 * qt * 512 : (2 * qt + 2) * 512],
                in0=wei[:, 2 * qt * 512 : (2 * qt + 2) * 512],
                in1=mask_sb[:],
            )
            # PV accumulate over slots
            pv = ps_pv.tile([H + 1, 512], fp32, tag="pv")
            for s in range(nkt):
                nc.tensor.matmul(
                    out=pv[:],
                    lhsT=v_sb[:, s, :],
                    rhs=wei[:, s * 512 : (s + 1) * 512],
                    start=(s == 0),
                    stop=(s == nkt - 1),
                    skip_group_check=True,
                )
            nc.vector.tensor_copy(out_sb[:, qt * 512 : (qt + 1) * 512], pv[:])
            nc.sync.dma_start(
                out=out_ext[:, qt * 512 : (qt + 1) * 512],
                in_=out_sb[:, qt * 512 : (qt + 1) * 512],
            )

    nc.compile()
    return nc


def _local_q_perm(p):
    """global query index for each local column (length T)."""
    perm = np.empty(T, dtype=np.int64)
    for qt in range(QT):
        blk_kts = [4 * qt + p, 4 * qt + 2 + p, 4 * qt + (1 - p), 4 * qt + 2 + (1 - p)]
        for i, kt in enumerate(blk_kts):
            lo = qt * 512 + i * 128
            perm[lo : lo + 128] = np.arange(kt * 128, kt * 128 + 128)
    return perm


def _build_masks(p):
    """[128, 2*512] bf16: the two stair slot masks (qt-independent in local
    order): si=0 -> [TRI,1,c,1], si=1 -> [0,TRI,0,c], c = (p == 0)."""
    r = np.arange(128)
    j = np.arange(128)
    tri = (r[:, None] <= j[None, :]).astype(np.float32)
    one = np.ones((128, 128), dtype=np.float32)
    zero = np.zeros((128, 128), dtype=np.float32)
    cblk = one if p == 0 else zero
    m0 = np.concatenate([tri, one, cblk, one], axis=1)
    m1 = np.concatenate([zero, tri, zero, cblk], axis=1)
    return np.ascontiguousarray(np.concatenate([m0, m1], axis=1)).astype(
        ml_dtypes.bfloat16
    )


def _make_in_maps(x, Wq, Wk, Wv, exchange=True):
    bf16 = ml_dtypes.bfloat16
    # [C, 128|H] -> SBUF layout [p=128, c=8, j]: row p holds chunk-c row c*128+p
    wqk = np.concatenate([Wq, Wk], axis=1).reshape(8, 128, 128)
    wqk = np.ascontiguousarray(wqk.transpose(1, 0, 2).reshape(128, 8 * 128)).astype(bf16)
    wv = Wv.reshape(8, 128, H)
    wv = np.ascontiguousarray(wv.transpose(1, 0, 2).reshape(128, 8 * H)).astype(bf16)
    in_maps = []
    for c in range(8):
        b, p = c // 2, c % 2
        own_kts = [2 * m + p for m in range(8)]
        peer_kts = [2 * m + (1 - p) for m in range(8)]
        kts = own_kts if exchange else own_kts + peer_kts
        rows = np.concatenate([np.arange(kt * 128, kt * 128 + 128) for kt in kts])
        xT_perm = np.ascontiguousarray(x[b][rows].T.astype(bf16))  # [C, 1024 or T]
        in_maps.append({"xT": xT_perm, "wqk": wqk, "wv": wv, "masks": _build_masks(p)})
    return in_maps


def _combine(per_core_out):
    """per_core_out: list of 8 arrays [H+1, T] (local query order) -> [B,T,H]."""
    out = np.empty((B, T, H), dtype=np.float32)
    for b in range(B):
        S = None
        for p in range(2):
            P_local = np.asarray(per_core_out[2 * b + p], dtype=np.float32)
            perm = _local_q_perm(p)
            P_glob = np.empty_like(P_local)
            P_glob[:, perm] = P_local
            S = P_glob if S is None else S + P_glob
        out[b] = (S[0:H, :] / S[H : H + 1, :]).T
    return out


EXCHANGE = os.environ.get("BASS_KERNEL_EXCHANGE", "1") == "1"


def kernel(x, Wq, Wk, Wv):
    global _COMPILED, LAST_EXEC_NS, LAST_RESULTS
    from concourse.bass_utils import run_bass_kernel_spmd

    x = np.ascontiguousarray(np.asarray(x, dtype=np.float32))
    Wq = np.asarray(Wq, dtype=np.float32)
    Wk = np.asarray(Wk, dtype=np.float32)
    Wv = np.asarray(Wv, dtype=np.float32)

    if _COMPILED is None:
        _COMPILED = _build_nc(exchange=EXCHANGE)
    nc = _COMPILED

    in_maps = _make_in_maps(x, Wq, Wk, Wv, exchange=EXCHANGE)
    trace = os.environ.get("BASS_KERNEL_TRACE", "0") == "1"
    res = run_bass_kernel_spmd(nc, in_maps, core_ids=list(range(8)), trace=trace)
    LAST_EXEC_NS = getattr(res, "exec_time_ns", None)
    LAST_RESULTS = res
    return _combine([res.results[c]["out"] for c in range(8)])
